# revision 15
# baseline (speedup 1.0000x reference)
"""Trainium2 Bass kernel for nn_MendGraph.

The op is almost pure data movement:
  fill_feats = concat(x, gen_feats.reshape(-1, F))          # 307 MB copy
  fill_edges = concat(edge_index, arange-based new edges)   # int copy
  edge_mask  = concat(ones(E), j < clip(round(pred_missing), 0, P))

Strategy: data-parallel over nodes across 8 NeuronCores. Each core
DRAM->DRAM-copies its shard of x/gen_feats/edge_index, generates its
candidate-edge block on-device (gpsimd iota + per-partition base add), and
computes its mask shard on the vector engine. Host only slices inputs and
stitches outputs.

The bulk copies are spread over all three descriptor feeds (SP HWDGE ring,
ACT HWDGE ring, gpsimd SWDGE ring): a single ring tops out well below the
per-core HBM ceiling (~660 GB/s r+w observed); with three rings the 16 SDMA
engines stay saturated. Small transfers use [25, wide] tiles so every
descriptor is >= 512 B (sub-512B HBM descriptors pay a read-modify-write
penalty). Output DMAs ride the SWDGE ring behind only 6.4 MB of x so they
land mid-window instead of at the kernel tail.

round-half-even (jnp.round) is reproduced exactly with per-slot strict
greater-than thresholds: round(pm) >= k  <=>  pm > t_k where t_k = k-0.5
for odd k (ties round down) and nextafter(k-0.5, -inf) for even k (ties
round up). Exact for any float32 pm.
"""

import sys

if "/opt/trn_rl_repo" not in sys.path:
    sys.path.insert(0, "/opt/trn_rl_repo")

import numpy as np

import concourse.bass as bass
import concourse.mybir as mybir
from concourse.bass_utils import run_bass_kernel_spmd

M = 8  # cores
N = 50000  # nodes
F = 256  # features
P = 5  # num_pred
E = 800000  # edges

NC_NODES = N // M  # 6250 nodes per core
NC_EDGES = E // M  # 100000 edges per core
NEWC = NC_NODES * P  # 31250 candidate edges per core

XW = NC_NODES * F  # x floats per core (1.6M)
GW = NC_NODES * P * F  # gen floats per core (8M)
GSPLIT = 5_000_000  # gen floats on the SP ring (20 MB); rest on ACT

SP_P = 25  # partitions for small tiles: descriptors >= 512B
PM_C = NC_NODES // SP_P  # 250
NEW_C = NEWC // SP_P  # 1250
ONES_C = NC_EDGES // SP_P  # 4000

# mask[i, j] = (round_half_even(pm[i]) >= j+1), exact thresholds (module doc)
_THRESH = [
    0.5,
    float(np.nextafter(np.float32(1.5), np.float32(0))),
    2.5,
    float(np.nextafter(np.float32(3.5), np.float32(0))),
    4.5,
]

_BUILD_CACHE = {}


def _build(ew: int) -> bass.Bass:
    """Per-core Bass program. ew = edge element size in int32 words
    (2 if edge_index is int64, 1 if int32)."""
    nc = bass.Bass()
    dt = mybir.dt

    x_in = nc.dram_tensor("x_in", [XW], dt.float32, kind="ExternalInput")
    gen_in = nc.dram_tensor("gen_in", [GW], dt.float32, kind="ExternalInput")
    pm_in = nc.dram_tensor("pm_in", [NC_NODES], dt.float32, kind="ExternalInput")
    eold_in = nc.dram_tensor(
        "eold_in", [2, NC_EDGES * ew], dt.int32, kind="ExternalInput"
    )
    # per-core global offsets: col0 = first node id, col1 = N + first candidate
    # id. float32: DVE reads AP scalar operands through a float path; values
    # are < 2^24 so exact.
    ebase_in = nc.dram_tensor("ebase_in", [SP_P, 2], dt.float32, kind="ExternalInput")

    feats_out = nc.dram_tensor(
        "feats_out", [XW + GW], dt.float32, kind="ExternalOutput"
    )
    eold_out = nc.dram_tensor(
        "eold_out", [2, NC_EDGES * ew], dt.int32, kind="ExternalOutput"
    )
    enew0_out = nc.dram_tensor(
        "enew0_out", [NEWC * ew], dt.int32, kind="ExternalOutput"
    )
    enew1_out = nc.dram_tensor(
        "enew1_out", [NEWC * ew], dt.int32, kind="ExternalOutput"
    )
    mones_out = nc.dram_tensor("mones_out", [NC_EDGES], dt.uint8, kind="ExternalOutput")
    mnew_out = nc.dram_tensor("mnew_out", [NEWC], dt.uint8, kind="ExternalOutput")

    n_gops = 2 * ew  # gpsimd compute ops: (memset +) iota per edge row
    n_vops = 1 + P + 2  # ones memset + 5 compares + 2 base adds

    with (
        nc.sbuf_tensor("pm_t", [SP_P, PM_C], dt.float32) as pm_t,
        nc.sbuf_tensor("mask_t", [SP_P, PM_C * P], dt.uint8) as mask_t,
        nc.sbuf_tensor("ones_t", [SP_P, ONES_C], dt.uint8) as ones_t,
        nc.sbuf_tensor("ebase_t", [SP_P, 2], dt.float32) as ebase_t,
        nc.sbuf_tensor("e0f_t", [SP_P, NEW_C], dt.float32) as e0f_t,
        nc.sbuf_tensor("e1f_t", [SP_P, NEW_C], dt.float32) as e1f_t,
        nc.sbuf_tensor("e0_t", [SP_P, NEW_C * ew], dt.int32) as e0_t,
        nc.sbuf_tensor("e1_t", [SP_P, NEW_C * ew], dt.int32) as e1_t,
        nc.semaphore("dsem1") as dsem1,  # sync-ring bulk DMA
        nc.semaphore("dsem2") as dsem2,  # scalar-ring bulk DMA
        nc.semaphore("dsem3") as dsem3,  # gpsimd-ring bulk DMA
        nc.semaphore("psem") as psem,  # small input DMAs
        nc.semaphore("gsem") as gsem,  # gpsimd iota done
        nc.semaphore("csem") as csem,  # vector compute done
        nc.semaphore("osem") as osem,  # output DMAs (gpsimd ring)
        nc.Block() as block,
    ):

        @block.sync
        def _(sync):
            sync.dma_start(
                out=feats_out[XW : XW + GSPLIT], in_=gen_in[0:GSPLIT]
            ).then_inc(dsem1, 16)
            sync.wait_ge(dsem1, 16)

        @block.scalar
        def _(scalar):
            # small inputs first (vector needs them), then bulk on the ACT ring
            scalar.dma_start(
                out=pm_t[:, :], in_=pm_in[:].rearrange("(p c) -> p c", p=SP_P)
            ).then_inc(psem, 16)
            scalar.dma_start(out=ebase_t[:, :], in_=ebase_in[:, :]).then_inc(psem, 16)
            scalar.dma_start(
                out=feats_out[XW + GSPLIT : XW + GW], in_=gen_in[GSPLIT:GW]
            ).then_inc(dsem2, 16)
            scalar.dma_start(out=eold_out[:, :], in_=eold_in[:, :]).then_inc(dsem2, 16)
            scalar.wait_ge(dsem2, 32)

        @block.gpsimd
        def _(gpsimd):
            # third DMA feed: x on the SWDGE ring
            gpsimd.dma_start(out=feats_out[0:XW], in_=x_in[:]).then_inc(dsem3, 16)
            # candidate edges: src = base0 + k//P, dst = base1 + k (k local).
            # iota in f32 (values <= 31249, exact); int64 output is int32
            # pairs [lo, 0]: zero the tile, DVE writes lo words strided.
            if ew == 2:
                gpsimd.memset(e0_t[:, :], 0).then_inc(gsem, 1)
                gpsimd.memset(e1_t[:, :], 0).then_inc(gsem, 1)
            gpsimd.iota(
                e0f_t[:, :],
                [[1, PM_C], [0, P]],
                channel_multiplier=PM_C,
                allow_small_or_imprecise_dtypes=True,
            ).then_inc(gsem, 1)
            gpsimd.iota(
                e1f_t[:, :],
                [[1, NEW_C]],
                channel_multiplier=NEW_C,
                allow_small_or_imprecise_dtypes=True,
            ).then_inc(gsem, 1)
            # outputs ride this ring behind only x, landing mid-window
            gpsimd.wait_ge(csem, n_vops)
            gpsimd.dma_start(
                out=mnew_out[:].rearrange("(p c) -> p c", p=SP_P), in_=mask_t[:, :]
            ).then_inc(osem, 16)
            gpsimd.dma_start(
                out=mones_out[:].rearrange("(p c) -> p c", p=SP_P), in_=ones_t[:, :]
            ).then_inc(osem, 16)
            gpsimd.dma_start(
                out=enew0_out[:].rearrange("(p c) -> p c", p=SP_P), in_=e0_t[:, :]
            ).then_inc(osem, 16)
            gpsimd.dma_start(
                out=enew1_out[:].rearrange("(p c) -> p c", p=SP_P), in_=e1_t[:, :]
            ).then_inc(osem, 16)
            gpsimd.wait_ge(dsem3, 16)
            gpsimd.wait_ge(osem, 64)

        @block.vector
        def _(vector):
            # gate all DVE work behind gpsimd (InstIota <-> DVE port-sharing
            # hazard) and the small-input DMAs
            vector.wait_ge(gsem, n_gops)
            vector.wait_ge(psem, 32)
            vector.memset(ones_t[:, :], 1).then_inc(csem, 1)
            e0_ap = e0_t[:, :: ew] if ew == 2 else e0_t[:, :]
            e1_ap = e1_t[:, :: ew] if ew == 2 else e1_t[:, :]
            vector.tensor_scalar(
                e0_ap, e0f_t[:, :], ebase_t[:, 0:1], None, mybir.AluOpType.add
            ).then_inc(csem, 1)
            vector.tensor_scalar(
                e1_ap, e1f_t[:, :], ebase_t[:, 1:2], None, mybir.AluOpType.add
            ).then_inc(csem, 1)
            for j in range(P):
                vector.tensor_scalar(
                    mask_t[:, j :: P],
                    pm_t[:, :],
                    _THRESH[j],
                    None,
                    mybir.AluOpType.is_gt,
                ).then_inc(csem, 1)

    return nc


def _get_nc(ew: int) -> bass.Bass:
    if ew not in _BUILD_CACHE:
        _BUILD_CACHE[ew] = _build(ew)
    return _BUILD_CACHE[ew]


def kernel(x, edge_index, pred_missing, gen_feats, num_pred=5):
    x = np.ascontiguousarray(np.asarray(x), dtype=np.float32)
    gen_feats = np.ascontiguousarray(np.asarray(gen_feats), dtype=np.float32)
    pred_missing = np.ascontiguousarray(np.asarray(pred_missing), dtype=np.float32)
    edge_index = np.ascontiguousarray(np.asarray(edge_index))

    edtype = edge_index.dtype
    ew = edtype.itemsize // 4
    assert ew in (1, 2), f"unexpected edge dtype {edtype}"
    ei32 = edge_index.view(np.int32)  # [2, E*ew]

    wc_old = NC_EDGES * ew
    wc_new = NEWC * ew
    in_maps = []
    for c in range(M):
        ebase = np.empty((SP_P, 2), dtype=np.float32)
        ebase[:, 0] = c * NC_NODES
        ebase[:, 1] = N + c * NEWC
        in_maps.append(
            {
                "x_in": x[c * NC_NODES : (c + 1) * NC_NODES].reshape(-1),
                "gen_in": gen_feats[c * NC_NODES : (c + 1) * NC_NODES].reshape(-1),
                "pm_in": pred_missing[c * NC_NODES : (c + 1) * NC_NODES],
                "eold_in": np.ascontiguousarray(
                    ei32[:, c * wc_old : (c + 1) * wc_old]
                ),
                "ebase_in": ebase,
            }
        )

    nc = _get_nc(ew)
    results = run_bass_kernel_spmd(nc, in_maps, list(range(M))).results

    # stitch full outputs
    fill_feats = np.empty((N + N * P, F), dtype=np.float32)
    fill_edges_32 = np.empty((2, (E + N * P) * ew), dtype=np.int32)
    edge_mask = np.empty(E + N * P, dtype=np.uint8)

    for c in range(M):
        r = results[c]
        fo = r["feats_out"]
        fill_feats[c * NC_NODES : (c + 1) * NC_NODES] = fo[:XW].reshape(NC_NODES, F)
        fill_feats[N + c * NEWC : N + (c + 1) * NEWC] = fo[XW:].reshape(NEWC, F)
        fill_edges_32[:, c * wc_old : (c + 1) * wc_old] = r["eold_out"]
        fill_edges_32[0, E * ew + c * wc_new : E * ew + (c + 1) * wc_new] = r[
            "enew0_out"
        ]
        fill_edges_32[1, E * ew + c * wc_new : E * ew + (c + 1) * wc_new] = r[
            "enew1_out"
        ]
        edge_mask[c * NC_EDGES : (c + 1) * NC_EDGES] = r["mones_out"]
        edge_mask[E + c * NEWC : E + (c + 1) * NEWC] = r["mnew_out"]

    fill_edges = fill_edges_32.view(edtype)
    return fill_feats, fill_edges, edge_mask.view(np.bool_)


# revision 18
# speedup vs baseline: 1.1777x; 1.1777x over previous
"""Trainium2 Bass kernel for nn_MendGraph.

The op is almost pure data movement:
  fill_feats = concat(x, gen_feats.reshape(-1, F))          # 307 MB copy
  fill_edges = concat(edge_index, arange-based new edges)   # int copy
  edge_mask  = concat(ones(E), j < clip(round(pred_missing), 0, P))

Strategy: data-parallel over nodes across 8 NeuronCores. Each core
DRAM->DRAM-copies its shard of x/gen_feats/edge_index, generates its
candidate-edge block on-device (gpsimd iota + per-partition base add), and
computes its mask shard on the vector engine. Host only slices inputs and
stitches outputs.

The bulk copies are spread over all three descriptor feeds (SP HWDGE ring,
ACT HWDGE ring, gpsimd SWDGE ring): a single ring tops out well below the
per-core HBM ceiling (~660 GB/s r+w observed); with three rings the 16 SDMA
engines stay saturated. Small transfers use [25, wide] tiles so every
descriptor is >= 512 B (sub-512B HBM descriptors pay a read-modify-write
penalty). Output DMAs ride the SWDGE ring behind only 6.4 MB of x so they
land mid-window instead of at the kernel tail.

round-half-even (jnp.round) is reproduced exactly with per-slot strict
greater-than thresholds: round(pm) >= k  <=>  pm > t_k where t_k = k-0.5
for odd k (ties round down) and nextafter(k-0.5, -inf) for even k (ties
round up). Exact for any float32 pm.
"""

import sys

if "/opt/trn_rl_repo" not in sys.path:
    sys.path.insert(0, "/opt/trn_rl_repo")

import numpy as np

import concourse.bass as bass
import concourse.mybir as mybir
from concourse.bass_utils import run_bass_kernel_spmd

M = 8  # cores
N = 50000  # nodes
F = 256  # features
P = 5  # num_pred
E = 800000  # edges

NC_NODES = N // M  # 6250 nodes per core
NC_EDGES = E // M  # 100000 edges per core
NEWC = NC_NODES * P  # 31250 candidate edges per core

XW = NC_NODES * F  # x floats per core (1.6M)
GW = NC_NODES * P * F  # gen floats per core (8M)
GSPLIT = 5_500_000  # gen floats on the SP ring (22 MB); rest on ACT

SP_P = 25  # partitions for small tiles: descriptors >= 512B
PM_C = NC_NODES // SP_P  # 250
NEW_C = NEWC // SP_P  # 1250
ONES_C = NC_EDGES // SP_P  # 4000

# mask[i, j] = (round_half_even(pm[i]) >= j+1), exact thresholds (module doc)
_THRESH = [
    0.5,
    float(np.nextafter(np.float32(1.5), np.float32(0))),
    2.5,
    float(np.nextafter(np.float32(3.5), np.float32(0))),
    4.5,
]

_BUILD_CACHE = {}


def _build(ew: int) -> bass.Bass:
    """Per-core Bass program. ew = edge element size in int32 words
    (2 if edge_index is int64, 1 if int32)."""
    nc = bass.Bass()
    dt = mybir.dt

    x_in = nc.dram_tensor("x_in", [XW], dt.float32, kind="ExternalInput")
    gen_in = nc.dram_tensor("gen_in", [GW], dt.float32, kind="ExternalInput")
    pm_in = nc.dram_tensor("pm_in", [NC_NODES], dt.float32, kind="ExternalInput")
    eold_in = nc.dram_tensor(
        "eold_in", [2, NC_EDGES * ew], dt.int32, kind="ExternalInput"
    )
    # per-core global offsets: col0 = first node id, col1 = N + first candidate
    # id. float32: DVE reads AP scalar operands through a float path; values
    # are < 2^24 so exact.
    ebase_in = nc.dram_tensor("ebase_in", [SP_P, 2], dt.float32, kind="ExternalInput")

    feats_out = nc.dram_tensor(
        "feats_out", [XW + GW], dt.float32, kind="ExternalOutput"
    )
    eold_out = nc.dram_tensor(
        "eold_out", [2, NC_EDGES * ew], dt.int32, kind="ExternalOutput"
    )
    enew0_out = nc.dram_tensor(
        "enew0_out", [NEWC * ew], dt.int32, kind="ExternalOutput"
    )
    enew1_out = nc.dram_tensor(
        "enew1_out", [NEWC * ew], dt.int32, kind="ExternalOutput"
    )
    mones_out = nc.dram_tensor("mones_out", [NC_EDGES], dt.uint8, kind="ExternalOutput")
    mnew_out = nc.dram_tensor("mnew_out", [NEWC], dt.uint8, kind="ExternalOutput")

    n_gops = 2 * ew  # gpsimd compute ops: (memset +) iota per edge row
    n_vops = 1 + P + 2  # ones memset + 5 compares + 2 base adds

    with (
        nc.sbuf_tensor("pm_t", [SP_P, PM_C], dt.float32) as pm_t,
        nc.sbuf_tensor("mask_t", [SP_P, PM_C * P], dt.uint8) as mask_t,
        nc.sbuf_tensor("ones_t", [SP_P, ONES_C], dt.uint8) as ones_t,
        nc.sbuf_tensor("ebase_t", [SP_P, 2], dt.float32) as ebase_t,
        nc.sbuf_tensor("e0f_t", [SP_P, NEW_C], dt.float32) as e0f_t,
        nc.sbuf_tensor("e1f_t", [SP_P, NEW_C], dt.float32) as e1f_t,
        nc.sbuf_tensor("e0_t", [SP_P, NEW_C * ew], dt.int32) as e0_t,
        nc.sbuf_tensor("e1_t", [SP_P, NEW_C * ew], dt.int32) as e1_t,
        nc.semaphore("dsem1") as dsem1,  # sync-ring bulk DMA
        nc.semaphore("dsem2") as dsem2,  # scalar-ring bulk DMA
        nc.semaphore("dsem3") as dsem3,  # gpsimd-ring bulk DMA
        nc.semaphore("psem") as psem,  # small input DMAs
        nc.semaphore("gsem") as gsem,  # gpsimd iota done
        nc.semaphore("csem") as csem,  # vector compute done
        nc.semaphore("osem") as osem,  # output DMAs (gpsimd ring)
        nc.Block() as block,
    ):

        @block.sync
        def _(sync):
            sync.dma_start(
                out=feats_out[XW : XW + GSPLIT], in_=gen_in[0:GSPLIT]
            ).then_inc(dsem1, 16)
            sync.wait_ge(dsem1, 16)

        @block.scalar
        def _(scalar):
            # small inputs first (vector needs them), then the outputs as soon
            # as compute is done (ahead of this ring's bulk, so they land
            # early instead of at the kernel tail), then bulk on the ACT ring
            scalar.dma_start(
                out=pm_t[:, :], in_=pm_in[:].rearrange("(p c) -> p c", p=SP_P)
            ).then_inc(psem, 16)
            scalar.dma_start(out=ebase_t[:, :], in_=ebase_in[:, :]).then_inc(psem, 16)
            scalar.wait_ge(csem, n_vops)
            scalar.dma_start(
                out=mnew_out[:].rearrange("(p c) -> p c", p=SP_P), in_=mask_t[:, :]
            ).then_inc(osem, 16)
            scalar.dma_start(
                out=mones_out[:].rearrange("(p c) -> p c", p=SP_P), in_=ones_t[:, :]
            ).then_inc(osem, 16)
            scalar.dma_start(
                out=enew0_out[:].rearrange("(p c) -> p c", p=SP_P), in_=e0_t[:, :]
            ).then_inc(osem, 16)
            scalar.dma_start(
                out=enew1_out[:].rearrange("(p c) -> p c", p=SP_P), in_=e1_t[:, :]
            ).then_inc(osem, 16)
            scalar.dma_start(
                out=feats_out[XW + GSPLIT : XW + GW], in_=gen_in[GSPLIT:GW]
            ).then_inc(dsem2, 16)
            scalar.dma_start(out=eold_out[:, :], in_=eold_in[:, :]).then_inc(dsem2, 16)
            scalar.wait_ge(dsem2, 32)
            scalar.wait_ge(osem, 64)

        @block.gpsimd
        def _(gpsimd):
            # third DMA feed: x on the SWDGE ring
            gpsimd.dma_start(out=feats_out[0:XW], in_=x_in[:]).then_inc(dsem3, 16)
            # candidate edges: src = base0 + k//P, dst = base1 + k (k local).
            # iota in f32 (values <= 31249, exact); int64 output is int32
            # pairs [lo, 0]: zero the tile, DVE writes lo words strided.
            if ew == 2:
                gpsimd.memset(e0_t[:, :], 0).then_inc(gsem, 1)
                gpsimd.memset(e1_t[:, :], 0).then_inc(gsem, 1)
            gpsimd.iota(
                e0f_t[:, :],
                [[1, PM_C], [0, P]],
                channel_multiplier=PM_C,
                allow_small_or_imprecise_dtypes=True,
            ).then_inc(gsem, 1)
            gpsimd.iota(
                e1f_t[:, :],
                [[1, NEW_C]],
                channel_multiplier=NEW_C,
                allow_small_or_imprecise_dtypes=True,
            ).then_inc(gsem, 1)
            gpsimd.wait_ge(dsem3, 16)

        @block.vector
        def _(vector):
            # gate all DVE work behind gpsimd (InstIota <-> DVE port-sharing
            # hazard) and the small-input DMAs
            vector.wait_ge(gsem, n_gops)
            vector.wait_ge(psem, 32)
            vector.memset(ones_t[:, :], 1).then_inc(csem, 1)
            e0_ap = e0_t[:, :: ew] if ew == 2 else e0_t[:, :]
            e1_ap = e1_t[:, :: ew] if ew == 2 else e1_t[:, :]
            vector.tensor_scalar(
                e0_ap, e0f_t[:, :], ebase_t[:, 0:1], None, mybir.AluOpType.add
            ).then_inc(csem, 1)
            vector.tensor_scalar(
                e1_ap, e1f_t[:, :], ebase_t[:, 1:2], None, mybir.AluOpType.add
            ).then_inc(csem, 1)
            for j in range(P):
                vector.tensor_scalar(
                    mask_t[:, j :: P],
                    pm_t[:, :],
                    _THRESH[j],
                    None,
                    mybir.AluOpType.is_gt,
                ).then_inc(csem, 1)

    return nc


def _get_nc(ew: int) -> bass.Bass:
    if ew not in _BUILD_CACHE:
        _BUILD_CACHE[ew] = _build(ew)
    return _BUILD_CACHE[ew]


def kernel(x, edge_index, pred_missing, gen_feats, num_pred=5):
    x = np.ascontiguousarray(np.asarray(x), dtype=np.float32)
    gen_feats = np.ascontiguousarray(np.asarray(gen_feats), dtype=np.float32)
    pred_missing = np.ascontiguousarray(np.asarray(pred_missing), dtype=np.float32)
    edge_index = np.ascontiguousarray(np.asarray(edge_index))

    edtype = edge_index.dtype
    ew = edtype.itemsize // 4
    assert ew in (1, 2), f"unexpected edge dtype {edtype}"
    ei32 = edge_index.view(np.int32)  # [2, E*ew]

    wc_old = NC_EDGES * ew
    wc_new = NEWC * ew
    in_maps = []
    for c in range(M):
        ebase = np.empty((SP_P, 2), dtype=np.float32)
        ebase[:, 0] = c * NC_NODES
        ebase[:, 1] = N + c * NEWC
        in_maps.append(
            {
                "x_in": x[c * NC_NODES : (c + 1) * NC_NODES].reshape(-1),
                "gen_in": gen_feats[c * NC_NODES : (c + 1) * NC_NODES].reshape(-1),
                "pm_in": pred_missing[c * NC_NODES : (c + 1) * NC_NODES],
                "eold_in": np.ascontiguousarray(
                    ei32[:, c * wc_old : (c + 1) * wc_old]
                ),
                "ebase_in": ebase,
            }
        )

    nc = _get_nc(ew)
    results = run_bass_kernel_spmd(nc, in_maps, list(range(M))).results

    # stitch full outputs
    fill_feats = np.empty((N + N * P, F), dtype=np.float32)
    fill_edges_32 = np.empty((2, (E + N * P) * ew), dtype=np.int32)
    edge_mask = np.empty(E + N * P, dtype=np.uint8)

    for c in range(M):
        r = results[c]
        fo = r["feats_out"]
        fill_feats[c * NC_NODES : (c + 1) * NC_NODES] = fo[:XW].reshape(NC_NODES, F)
        fill_feats[N + c * NEWC : N + (c + 1) * NEWC] = fo[XW:].reshape(NEWC, F)
        fill_edges_32[:, c * wc_old : (c + 1) * wc_old] = r["eold_out"]
        fill_edges_32[0, E * ew + c * wc_new : E * ew + (c + 1) * wc_new] = r[
            "enew0_out"
        ]
        fill_edges_32[1, E * ew + c * wc_new : E * ew + (c + 1) * wc_new] = r[
            "enew1_out"
        ]
        edge_mask[c * NC_EDGES : (c + 1) * NC_EDGES] = r["mones_out"]
        edge_mask[E + c * NEWC : E + (c + 1) * NEWC] = r["mnew_out"]

    fill_edges = fill_edges_32.view(edtype)
    return fill_feats, fill_edges, edge_mask.view(np.bool_)
